# revision 17
# baseline (speedup 1.0000x reference)
"""Swin-style attention (B=64,N=512,C=768,H=12) on 8 TRN2 NeuronCores.

Strategy: pure data-parallel over batch (8 batches/core), no collectives.
Per core, one fused pipeline per batch:
  phase1: qkT = Wqk @ x^T (bf16 matmuls), v = x @ Wv^T (natural layout,
          padded with a ones-column per head for fused softmax sums)
  attn:   per head pair (hA rows 0:63, hB rows 64:127 of the PE array):
          score quads sq[:,0,:]=A, sq[:,1,:]=B issued back-to-back so the
          two K=64 matmuls execute CONCURRENTLY on disjoint row groups
          (verified: pairs start ~4ns apart, 2 matmuls per 216ns slot);
          one 1024-wide exp per quad on ACT; bias multiply on DVE;
          oT~[d,i] & sums via [v|1] matmul (bf16, M=65); row sums
          broadcast via a DRAM round-trip; normalize on DVE in bf16.
  proj:   out = oT^T @ Wp^T + pb (bf16 matmul)
Scale 1/8 is folded into the q-half of Wqk on the host; softmax runs
without max-subtraction (scores are O(1) by construction).
A ~10-matmul dummy warmup burst trips the PE HAM clock gate to 2.4 GHz
during the initial DMA fill, so real work starts at full clock.
"""
import sys

sys.path.insert(0, "/opt/trn_rl_repo")
from contextlib import ExitStack

import ml_dtypes
import numpy as np

import concourse.bass as bass
import concourse.mybir as mybir
import concourse.tile as tile
from concourse import bacc
from concourse.bass_utils import run_bass_kernel_spmd

F32 = mybir.dt.float32
BF16 = mybir.dt.bfloat16

B, N, C, H, HD = 64, 512, 768, 12, 64
NCORES = 8
BL = B // NCORES          # batches per core
T = BL * N                # tokens per core
KC = C // 128             # 6 contraction chunks
NJT = N // 128            # 4 key-side tiles
NIT = N // 128            # 4 query/token tiles
VP = H * (HD + 1)         # 780: v padded with ones column per head
Exp = mybir.ActivationFunctionType.Exp


PB_IS_ZERO = True


def _build():
    nc = bacc.Bacc(target_bir_lowering=False)
    xT_d = nc.dram_tensor("xT", [C, T], BF16, kind="ExternalInput")
    wqk_d = nc.dram_tensor("wqk", [C, 2 * C], BF16, kind="ExternalInput")
    wv_d = nc.dram_tensor("wv", [C, C], BF16, kind="ExternalInput")
    wp_d = nc.dram_tensor("wp", [C, C], BF16, kind="ExternalInput")
    biasT_d = nc.dram_tensor("biasT", [H // 2 * NJT, 2, 128, N], BF16, kind="ExternalInput")
    pb_d = nc.dram_tensor("pb", [1, C], F32, kind="ExternalInput")
    out_d = nc.dram_tensor("out", [T, C], BF16, kind="ExternalOutput")

    with ExitStack() as ctx:
        tc = ctx.enter_context(tile.TileContext(nc))
        const = ctx.enter_context(tc.tile_pool(name="const", bufs=1))
        perb = ctx.enter_context(tc.tile_pool(name="perb", bufs=2))
        perb1 = ctx.enter_context(tc.tile_pool(name="perb1", bufs=2))
        xt_pool = ctx.enter_context(tc.tile_pool(name="xt", bufs=2))
        pool_pe = ctx.enter_context(tc.tile_pool(name="pe", bufs=3))
        pool_p = ctx.enter_context(tc.tile_pool(name="pt", bufs=2))
        pool_r = ctx.enter_context(tc.tile_pool(name="rc", bufs=2))
        pool_o = ctx.enter_context(tc.tile_pool(name="osb", bufs=2))
        dram_p = ctx.enter_context(tc.tile_pool(name="dramp", bufs=2, space="DRAM"))
        mm_ps = ctx.enter_context(tc.tile_pool(name="mmps", bufs=2, space="PSUM"))
        s_ps = ctx.enter_context(tc.tile_pool(name="sps", bufs=2, space="PSUM"))
        o_ps = ctx.enter_context(tc.tile_pool(name="ops", bufs=2, space="PSUM"))

        # ---- constants ----
        wqk = const.tile([128, KC, 2 * C], BF16)
        wv = const.tile([128, KC, C], BF16)
        wp = const.tile([128, KC, C], BF16)
        biasT = const.tile([128, H // 2 * NJT, 2, N], BF16)
        pb_bc = const.tile([128, C], F32)
        warm = const.tile([128, N], BF16)

        def load_consts_late():
            for kc in range(KC):
                nc.sync.dma_start(
                    out=wp[:, kc, :], in_=wp_d[kc * 128:(kc + 1) * 128, :]
                )
            for q in range(H // 2 * NJT):
                nc.sync.dma_start(
                    out=biasT[:, q, :, :],
                    in_=biasT_d[q, :, :, :].rearrange("a p b -> p a b"),
                )
            if not PB_IS_ZERO:
                nc.sync.dma_start(
                    out=pb_bc, in_=pb_d[0:1, :].to_broadcast((128, C))
                )

        def load_x(b):
            xTb = xt_pool.tile([128, KC, N], BF16, tag="xTb")
            for kc in range(KC):
                nc.sync.dma_start(
                    out=xTb[:, kc, :],
                    in_=xT_d[kc * 128:(kc + 1) * 128, b * N:(b + 1) * N],
                )
            return xTb

        def alloc_qkT():
            return perb.tile([128, 2 * H // 2, N], BF16, tag="qkT", name="qkT")

        def alloc_vpad():
            v_pad = perb.tile([128, NIT, VP], BF16, tag="v_pad")
            ones_view = v_pad.rearrange("p a (h e) -> p (a h) e", e=HD + 1)
            nc.vector.memset(ones_view[:, :, HD:HD + 1], 1.0)
            return v_pad

        def qk_tile(qkT, xTb, rt):
            ps = mm_ps.tile([128, N], F32, tag="mm")
            for kc in range(KC):
                nc.tensor.matmul(
                    ps,
                    wqk[:, kc, rt * 128:(rt + 1) * 128],
                    xTb[:, kc, :],
                    start=(kc == 0),
                    stop=(kc == KC - 1),
                )
            nc.vector.tensor_copy(out=qkT[:, rt, :], in_=ps)

        def v_tile(v_pad, xTb, it, nh):
            ps = mm_ps.tile([128, C // 2], F32, tag="mm")
            for kc in range(KC):
                nc.tensor.matmul(
                    ps,
                    xTb[:, kc, it * 128:(it + 1) * 128],
                    wv[:, kc, nh * 384:(nh + 1) * 384],
                    start=(kc == 0),
                    stop=(kc == KC - 1),
                )
            dest = v_pad[:, it, :].rearrange("p (h e) -> p h e", e=HD + 1)
            nc.vector.tensor_copy(
                out=dest[:, nh * 6:(nh + 1) * 6, 0:HD],
                in_=ps.rearrange("p (h e) -> p h e", e=HD),
            )

        def proj_tile(b, oT, it):
            outsb = pool_o.tile([128, C], BF16, tag="outsb")
            for ct in range(2):
                ps = mm_ps.tile([128, C // 2], F32, tag="mm")
                for kc in range(KC):
                    nc.tensor.matmul(
                        ps,
                        oT[:, kc, it * 128:(it + 1) * 128],
                        wp[:, kc, ct * 384:(ct + 1) * 384],
                        start=(kc == 0),
                        stop=(kc == KC - 1),
                    )
                if PB_IS_ZERO:
                    nc.scalar.copy(out=outsb[:, ct * 384:(ct + 1) * 384], in_=ps)
                else:
                    nc.vector.tensor_add(
                        outsb[:, ct * 384:(ct + 1) * 384],
                        ps,
                        pb_bc[:, ct * 384:(ct + 1) * 384],
                    )
            nc.sync.dma_start(
                out=out_d[b * N + it * 128: b * N + (it + 1) * 128, :],
                in_=outsb,
            )

        def quad_scores(qkT, hp, jt):
            # head pair (hA=2hp -> PE rows 0-63, hB=2hp+1 -> rows 64-127):
            # the two K=64 score matmuls are issued back-to-back into one
            # psum quad so the hardware runs them concurrently on disjoint
            # row groups of the systolic array.
            rq, rk = hp, H // 2 + hp
            sq = s_ps.tile([128, 2, N], F32, tag="sq")
            nc.tensor.matmul(
                sq[:, 0, :],
                qkT[0:64, rk, jt * 128:(jt + 1) * 128],
                qkT[0:64, rq, :],
                start=True,
                stop=True,
            )
            nc.tensor.matmul(
                sq[:, 1, :],
                qkT[64:128, rk, jt * 128:(jt + 1) * 128],
                qkT[64:128, rq, :],
                start=True,
                stop=True,
            )
            return sq

        def quad_exp_mul(sq, hp, jt):
            # one 1024-wide exp per quad (ACT), then ONE 1024-wide bias
            # multiply per quad (DVE, bf16 2x) against the head-interleaved
            # pre-exponentiated bias table.  High priority jumps these ahead
            # of filler copies in the ACT/DVE queues: the exp gates score
            # psum recycling (quad t+2 WARs exp t), the mul gates attnv.
            pe = pool_pe.tile([128, 2, N], BF16, tag="pe")
            pt = pool_p.tile([128, 2, N], BF16, tag=f"pt{jt}")
            with tc.high_priority(offset=40):
                nc.scalar.activation(out=pe, in_=sq, func=Exp)
                nc.vector.tensor_mul(pt, pe, biasT[:, hp * NJT + jt, :, :])
            return pt

        def head_out(oT, v_pad, h, ptq):
            po = (h % 2) * 64
            rqo = h // 2
            pso = o_ps.tile([HD + 1, N], F32, tag="oT")
            for jt in range(NJT):
                vp = v_pad[:, jt, :].rearrange("p (h e) -> p h e", e=HD + 1)
                nc.tensor.matmul(
                    pso,
                    vp[:, h, :],
                    ptq[jt][:, h % 2, :],
                    start=(jt == 0),
                    stop=(jt == NJT - 1),
                )
            nc.vector.tensor_copy(out=oT[po:po + 64, rqo, :], in_=pso[0:HD, :])
            smc = pool_r.tile([1, N], F32, tag="smc")
            nc.scalar.copy(out=smc, in_=pso[HD:HD + 1, :])
            rcd = dram_p.tile([1, N], F32, tag="rcd")
            nc.sync.dma_start(out=rcd, in_=smc)
            return rcd

        def pair_norm(oT, hp, rcdA, rcdB):
            # one [128,N] tile: sums(A) on partitions 0-63, sums(B) on
            # 64-127 -> a single reciprocal (bf16 out) + two bf16 multiplies.
            rqo = hp
            rcf = pool_r.tile([128, N], F32, tag="rcf")
            nc.sync.dma_start(out=rcf[0:64, :], in_=rcdA[0:1, :].to_broadcast((64, N)))
            nc.sync.dma_start(out=rcf[64:128, :], in_=rcdB[0:1, :].to_broadcast((64, N)))
            nc.vector.reciprocal_approx_fast(out=rcf, in_=rcf)
            nc.gpsimd.tensor_mul(
                oT[0:64, rqo, :], oT[0:64, rqo, :], rcf[0:64, :]
            )
            nc.gpsimd.tensor_mul(
                oT[64:128, rqo, :], oT[64:128, rqo, :], rcf[64:128, :]
            )

        # ---- software-pipelined schedule ----
        # A persistent filler queue carries the next batch's qkv tiles and
        # the previous batch's projection tiles; pairs pop fillers between
        # their score and o matmuls, and dependencies are force-drained
        # just in time.  This keeps the PE dense through the last batch.
        work = []  # list of (key, fn); key=(kind, b, a, c)

        def drain(pred):
            i = 0
            while i < len(work):
                if pred(work[i][0]):
                    work.pop(i)[1]()
                else:
                    i += 1

        def pop_one():
            if work:
                work.pop(0)[1]()

        # warmup: dummy matmuls with no input dependencies trip the PE HAM
        # clock gate to 8/8 during the initial DMA fill (~4.3us of cold-rate
        # matmul occupancy; the first real matmul can't start earlier anyway).
        nc.gpsimd.memset(warm, 0.0)

        def warm_burst(n):
            warm_ps = mm_ps.tile([128, N], F32, tag="mm")
            for i in range(n):
                nc.tensor.matmul(warm_ps, warm[:, 0:128], warm, start=True,
                                 stop=True)
        warm_burst(10)

        # startup: interleave wqk/x(0) chunk loads so the first matmul can
        # begin after one chunk of each.
        xT_cur = xt_pool.tile([128, KC, N], BF16, tag="xTb", name="xT0")
        for kc in range(KC):
            nc.sync.dma_start(
                out=wqk[:, kc, :], in_=wqk_d[kc * 128:(kc + 1) * 128, :]
            )
            nc.sync.dma_start(
                out=xT_cur[:, kc, :], in_=xT_d[kc * 128:(kc + 1) * 128, 0:N]
            )
        for kc in range(KC):
            nc.sync.dma_start(out=wv[:, kc, :], in_=wv_d[kc * 128:(kc + 1) * 128, :])
        qkT_cur = alloc_qkT()
        vp_cur = alloc_vpad()
        for rt in range(12):
            qk_tile(qkT_cur, xT_cur, rt)
        load_consts_late()
        for it in range(NIT):
            for nh in range(2):
                v_tile(vp_cur, xT_cur, it, nh)

        oT_prev, b_prev = None, None
        deferred = []
        for b in range(BL):
            qkT, v_pad = qkT_cur, vp_cur
            fillers = list(deferred)
            deferred = []
            if b + 1 < BL:
                xT_nxt = load_x(b + 1)
                qkT_cur = alloc_qkT()
                vp_cur = alloc_vpad()
                if b + 1 == BL - 1:
                    # the last batch has no successor to fill its pair gaps:
                    # run only the tiles its first pairs need now, defer the
                    # rest (in dependency-safe order) as its own fillers.
                    early_rt = [0, 6, 1, 7]
                    late_rt = [2, 8, 3, 9, 4, 10, 5, 11]
                    fillers += [
                        (lambda rt=rt, q=qkT_cur, x=xT_nxt: qk_tile(q, x, rt))
                        for rt in early_rt
                    ]
                    fillers += [
                        (lambda it=it, v=vp_cur, x=xT_nxt: v_tile(v, x, it, 0))
                        for it in range(NIT)
                    ]
                    if oT_prev is not None:
                        fillers += [
                            (lambda it=it, ob=b_prev, ot=oT_prev:
                             proj_tile(ob, ot, it))
                            for it in range(NIT)
                        ]
                    deferred += [
                        (lambda rt=rt, q=qkT_cur, x=xT_nxt: qk_tile(q, x, rt))
                        for rt in late_rt[:6]
                    ]
                    deferred += [
                        (lambda it=it, v=vp_cur, x=xT_nxt: v_tile(v, x, it, 1))
                        for it in range(2)
                    ]
                    deferred += [
                        (lambda rt=rt, q=qkT_cur, x=xT_nxt: qk_tile(q, x, rt))
                        for rt in late_rt[6:]
                    ]
                    deferred += [
                        (lambda it=it, v=vp_cur, x=xT_nxt: v_tile(v, x, it, 1))
                        for it in range(2, NIT)
                    ]
                else:
                    fillers += [
                        (lambda rt=rt, q=qkT_cur, x=xT_nxt: qk_tile(q, x, rt))
                        for rt in range(12)
                    ]
                    fillers += [
                        (lambda it=it, nh=nh, v=vp_cur, x=xT_nxt:
                         v_tile(v, x, it, nh))
                        for it in range(NIT) for nh in range(2)
                    ]
                    if oT_prev is not None:
                        fillers += [
                            (lambda it=it, ob=b_prev, ot=oT_prev:
                             proj_tile(ob, ot, it))
                            for it in range(NIT)
                        ]
            else:
                if oT_prev is not None:
                    fillers += [
                        (lambda it=it, ob=b_prev, ot=oT_prev:
                         proj_tile(ob, ot, it))
                        for it in range(NIT)
                    ]
            # spread fillers evenly over the 24 pair slots; the last batch's
            # fillers carry intra-batch dependencies, so keep their order and
            # pack them densely from the front instead.
            slots = [None] * 24
            nf = len(fillers)
            if nf:
                if b == BL - 1:
                    # first 12 (dependency-ordered) pack densely; the rest
                    # (projection tiles, order-free) go to the starved tail
                    # pairs.
                    for i, f in enumerate(fillers[:12]):
                        slots[i] = f
                    for i, f in zip((12, 15, 18, 21), fillers[12:16]):
                        slots[i] = f
                else:
                    for i, f in enumerate(fillers[:24]):
                        slots[(i * 24) // min(nf, 24)] = f
            extra = fillers[24:]

            def use(si):
                if slots[si] is not None:
                    slots[si]()

            oT = perb1.tile([128, KC, N], BF16, tag="oT")
            for hp in range(H // 2):
                ptq = [None] * NJT
                sq0 = quad_scores(qkT, hp, 0)
                ptq[0] = quad_exp_mul(sq0, hp, 0)
                sq1 = quad_scores(qkT, hp, 1)
                ptq[1] = quad_exp_mul(sq1, hp, 1)
                use(hp * 4)
                sq2 = quad_scores(qkT, hp, 2)
                ptq[2] = quad_exp_mul(sq2, hp, 2)
                sq3 = quad_scores(qkT, hp, 3)
                ptq[3] = quad_exp_mul(sq3, hp, 3)
                use(hp * 4 + 1)
                rcdA = head_out(oT, v_pad, 2 * hp, ptq)
                use(hp * 4 + 2)
                rcdB = head_out(oT, v_pad, 2 * hp + 1, ptq)
                use(hp * 4 + 3)
                pair_norm(oT, hp, rcdA, rcdB)
            for f in extra:
                f()
            oT_prev, b_prev = oT, b
        for it in range(NIT):
            proj_tile(b_prev, oT_prev, it)
    nc.finalize()
    return nc


def kernel(x, qkv_w, proj_w, proj_b, bias_table, _trace=False, _tmpdir=None):
    x = np.asarray(x, dtype=np.float32)
    qkv_w = np.asarray(qkv_w, dtype=np.float32)
    proj_w = np.asarray(proj_w, dtype=np.float32)
    proj_b = np.asarray(proj_b, dtype=np.float32)
    bias_table = np.asarray(bias_table, dtype=np.float32)

    # host-side layout prep (weights + bias table expansion)
    wq_scaled = qkv_w.copy()
    wq_scaled[:C] *= HD ** (-0.5)
    wqk = np.ascontiguousarray(wq_scaled[: 2 * C].T).astype(ml_dtypes.bfloat16)
    wv = np.ascontiguousarray(qkv_w[2 * C:].T).astype(ml_dtypes.bfloat16)
    wp = np.ascontiguousarray(proj_w.T).astype(ml_dtypes.bfloat16)
    ii = np.arange(N)
    idx = ii[None, :] - ii[:, None] + (N - 1)                     # [j, i]
    ebT = np.exp(bias_table[idx].transpose(2, 0, 1)).reshape(H, NJT, 128, N)
    biasQ = np.empty((H // 2, NJT, 2, 128, N), dtype=np.float32)
    biasQ[:, :, 0] = ebT[0::2]
    biasQ[:, :, 1] = ebT[1::2]
    biasT = np.ascontiguousarray(
        biasQ.reshape(H // 2 * NJT, 2, 128, N)
    ).astype(ml_dtypes.bfloat16)
    pb = proj_b.reshape(1, C)

    global PB_IS_ZERO
    PB_IS_ZERO = not np.any(proj_b)
    nc = _build()
    in_maps = []
    for m in range(NCORES):
        xs = x[m * BL:(m + 1) * BL]                               # [8, 512, 768]
        xT = np.ascontiguousarray(xs.transpose(2, 0, 1).reshape(C, T)).astype(ml_dtypes.bfloat16)
        in_maps.append(
            {"xT": xT, "wqk": wqk, "wv": wv, "wp": wp, "biasT": biasT, "pb": pb}
        )
    res = run_bass_kernel_spmd(
        nc, in_maps, core_ids=list(range(NCORES)), trace=_trace, tmpdir=_tmpdir
    )
    out = np.concatenate(
        [np.asarray(res.results[m]["out"], dtype=np.float32).reshape(BL, N, C)
         for m in range(NCORES)], axis=0
    )
    if _trace:
        return out, res
    return out
